# revision 12
# baseline (speedup 1.0000x reference)
"""DropGraph Trainium2 kernel (nn_DropGraph_24713241822120).

out[b,c,t,n] = x[b,c,t,n] * mask[b,n] / mean(mask), where mask[b,n] zeroes the
adjacency neighborhood of seed_idx[b] when drop_rand[b] < 0.1.

Strategy: the mask/denominator depend only on the tiny [B]/[B,N] inputs, so they
are computed on host and folded into a per-(batch,node) scale tensor. The device
work is the memory-bound part: stream all of x (400MB) through the 8 NeuronCores
(batch-sharded, 8 batches per core) and multiply by the scale, broadcast over
the C and T axes. Layout per batch slab: [C=128 partitions, T*N=12288 free]
(contiguous in HBM), multiplied in-place by a [C, N] scale tile whose access
pattern repeats T times via a stride-0 middle dim.
"""

import sys

if "/opt/trn_rl_repo" not in sys.path:
    sys.path.insert(0, "/opt/trn_rl_repo")

import numpy as np

# Problem constants (hardcoded per harness contract).
B, C, T, N = 64, 128, 256, 48
NCORES = 8
BL = B // NCORES  # batches per core
P_DROP = 0.1

HAND_EDGES = [
    (0, 1), (0, 5), (0, 9), (0, 13), (0, 17), (1, 2), (2, 3), (3, 4),
    (5, 6), (6, 7), (7, 8), (9, 10), (10, 11), (11, 12), (13, 14),
    (14, 15), (15, 16), (17, 18), (18, 19), (19, 20), (5, 9), (9, 13),
    (13, 17),
]
POSE_EDGES = [(42, 43), (42, 44), (43, 45), (44, 46), (45, 47), (46, 0), (47, 21)]


def _build_adjacency(n=N):
    adj = np.zeros((n, n), dtype=bool)
    edges = list(HAND_EDGES) + [(i + 21, j + 21) for i, j in HAND_EDGES] + list(POSE_EDGES)
    for i, j in edges:
        adj[i, j] = True
        adj[j, i] = True
    adj[np.arange(n), np.arange(n)] = True
    return adj


ADJ = _build_adjacency()

_NC = None


def _build_bass(passes=1, t_split=4, bufs=None, ring_mix=True, gp_every=0, ring3=False):
    """Build the per-core Bass module once. Structure is input-independent.

    passes>1 repeats the whole streaming body (same I/O) — used only by the
    timing harness to isolate device time from dispatch overhead via slope.
    t_split splits each batch slab into chunks along T (finer pipelining;
    chunk DMAs stay >=3MB for t_split<=2, keeping HBM the binding resource).
    """
    import concourse.bacc as bacc
    import concourse.mybir as mybir
    from concourse import tile

    assert T % t_split == 0
    tc_len = (T // t_split) * N  # free elems per chunk
    if bufs is None:
        bufs = 3 * t_split  # same total SBUF as 3 full-slab buffers

    nc = bacc.Bacc("TRN2", target_bir_lowering=False)
    f32 = mybir.dt.float32
    x = nc.dram_tensor("x", [BL, C, T * N], f32, kind="ExternalInput")
    s = nc.dram_tensor("s", [C, BL, N], f32, kind="ExternalInput")
    y = nc.dram_tensor("y", [BL, C, T * N], f32, kind="ExternalOutput")

    with tile.TileContext(nc) as tc:
        with (
            tc.tile_pool(name="xp", bufs=bufs) as xp,
            tc.tile_pool(name="sp", bufs=1) as sp,
        ):
            st = sp.tile([C, BL * N], f32)
            nc.sync.dma_start(out=st[:, :], in_=s[:, :, :].rearrange("c b n -> c (b n)"))
            for _ in range(passes):
                for b in range(BL):
                    s3 = (
                        st[:, b * N : (b + 1) * N]
                        .unsqueeze(1)
                        .broadcast_to([C, T // t_split, N])
                    )
                    for k in range(t_split):
                        lo = k * tc_len
                        # Ring policy: alternate the two HWDGE rings (SP/ACT)
                        # per chunk so loads and stores each draw on both
                        # descriptor streams; ring3 adds the SWDGE queue
                        # (gpsimd) as a third stream; or pin loads=SP /
                        # stores=ACT.
                        ci3 = b * t_split + k
                        if ring3:
                            rot = [
                                (nc.sync, nc.scalar),
                                (nc.scalar, nc.gpsimd),
                                (nc.gpsimd, nc.sync),
                            ]
                            ld, stq = rot[ci3 % 3]
                        elif ring_mix:
                            ld = nc.sync if ci3 % 2 == 0 else nc.scalar
                            stq = nc.scalar if ci3 % 2 == 0 else nc.sync
                        else:
                            ld, stq = nc.sync, nc.scalar
                        xt = xp.tile([C, tc_len], f32)
                        ld.dma_start(out=xt[:, :], in_=x[b, :, lo : lo + tc_len])
                        x3 = xt[:, :].rearrange("c (t n) -> c t n", n=N)
                        # Optionally route every gp_every-th chunk's multiply
                        # to GPSIMD (Pool) to relieve the DVE.
                        ci = b * t_split + k
                        eng = (
                            nc.gpsimd
                            if gp_every and ci % gp_every == gp_every - 1
                            else nc.vector
                        )
                        eng.tensor_mul(out=x3, in0=x3, in1=s3)
                        stq.dma_start(out=y[b, :, lo : lo + tc_len], in_=xt[:, :])
    nc.compile()
    return nc


def _get_nc():
    global _NC
    if _NC is None:
        _NC = _build_bass()
    return _NC


def _make_in_maps(np_inputs):
    """Host-side prep: mask + keep-ratio folded into a per-(batch,node) scale,
    inputs sharded along batch across the 8 cores."""
    x = np.ascontiguousarray(np.asarray(np_inputs["x"], dtype=np.float32))
    drop_rand = np.asarray(np_inputs["drop_rand"], dtype=np.float32)
    seed_idx = np.asarray(np_inputs["seed_idx"]).astype(np.int64)

    # Mirrors the f32 reference math: the mask sum is an exact small integer
    # in f32, so the mean is bit-identical to jnp.mean.
    drop = drop_rand < np.float32(P_DROP)                      # [B]
    dropped = ADJ[seed_idx] & drop[:, None]                    # [B, N]
    mask = (~dropped).astype(np.float32)                       # [B, N]
    keep_ratio = np.float32(mask.sum(dtype=np.float64)) / np.float32(B * N)
    denom = keep_ratio if keep_ratio > 0 else np.float32(1.0)
    scale = (mask / denom).astype(np.float32)                  # [B, N]

    in_maps = []
    for c in range(NCORES):
        xs = x[c * BL : (c + 1) * BL].reshape(BL, C, T * N)
        ss = np.ascontiguousarray(
            np.broadcast_to(scale[c * BL : (c + 1) * BL][None, :, :], (C, BL, N))
        )
        in_maps.append({"x": xs, "s": ss})
    return in_maps, scale


def kernel(x, drop_rand, seed_idx):
    from concourse.bass_utils import run_bass_kernel_spmd

    in_maps, _ = _make_in_maps(
        {"x": x, "drop_rand": drop_rand, "seed_idx": seed_idx}
    )
    nc = _get_nc()
    res = run_bass_kernel_spmd(nc, in_maps, core_ids=list(range(NCORES)))
    out = np.concatenate(
        [r["y"].reshape(BL, C, T, N) for r in res.results], axis=0
    )
    return out
